# revision 9
# baseline (speedup 1.0000x reference)
"""Trainium2 Bass kernel for nn_CovDiFF_full_40312563040970.

Pipeline (per the reference model):
  img branch: 3x3 conv (+folded BN) + ReLU -> Z  [128, 307200]
  pcd branch: linear + ReLU               -> Zp [128, 22432]
  cov_img = cov(Z), cov_pcd = cov(Zp)  (torch.cov semantics)
  attention matrices from covariances (inverse + row-softmax), then
  out = TAU/2*(at_a + at_b) @ Z  + (1-TAU)*input   for each branch.

Distribution: 8 NeuronCores; image rows and points sharded across cores.
Each core computes a partial covariance (sum-of-outer-products + row sums),
a 128x258 AllReduce combines them, and the tiny 128x128 math (Newton-Schulz
inverse, softmax) is replicated on every core.  The big C x N matmuls
(conv-as-9-matmuls, cov accumulation, final mixing matmul) run on the
TensorEngine in float32r (full-rate) / bf16.
"""

import numpy as np
import ml_dtypes

P = 128
H, W = 480, 640
NCORES = 8
RPC = H // NCORES            # 60 output rows per core
NPPC = RPC * W               # 38400 pixels per core
NPIX = H * W                 # 307200 pixels total
NP = 22432                   # points total
NPPC_P = NP // NCORES        # 2804 points per core
PCHUNKS = 22                 # point chunks of 128 (padded)
NP_PAD = PCHUNKS * P         # 2816
TAU = 0.2
BN_EPS = 1e-5
EYE_EPS = 1e-6
NS_ITERS = 14
ROW_CHUNK = 10               # conv: output rows per input DMA chunk
CONV_N = 320                 # conv matmul free dim (640 = 2x320)

_cache = {}


def _build_program():
    import concourse.bacc as bacc
    import concourse.bass as bass
    import concourse.tile as tile
    import concourse.mybir as mybir
    from concourse import bass_isa
    from concourse.bass import ts

    f32 = mybir.dt.float32
    f32r = mybir.dt.float32r
    bf16 = mybir.dt.bfloat16
    AX = mybir.AxisListType
    OP = mybir.AluOpType
    AF = mybir.ActivationFunctionType

    nc = bacc.Bacc(
        "TRN2", target_bir_lowering=False, debug=False, num_devices=NCORES
    )

    img = nc.dram_tensor("img", [P, RPC + 2, W + 2], f32r, kind="ExternalInput").ap()
    pcd = nc.dram_tensor("pcd", [NP_PAD, P], f32, kind="ExternalInput").ap()
    w9t = nc.dram_tensor("w9t", [9, P, P], f32r, kind="ExternalInput").ap()
    cbv = nc.dram_tensor("cb", [P, 1], f32, kind="ExternalInput").ap()
    lwt = nc.dram_tensor("lwt", [P, P], f32r, kind="ExternalInput").ap()
    lbv = nc.dram_tensor("lb", [P, 1], f32, kind="ExternalInput").ap()
    idf = nc.dram_tensor("idf", [P, P], f32, kind="ExternalInput").ap()
    idb = nc.dram_tensor("idb", [P, P], bf16, kind="ExternalInput").ap()

    img_out = nc.dram_tensor("img_out", [P, RPC, W], f32, kind="ExternalOutput").ap()
    pcd_out = nc.dram_tensor("pcd_out", [NP_PAD, P], f32, kind="ExternalOutput").ap()

    cc_in = nc.dram_tensor("cc_in", [P, 258], f32, kind="Internal").ap()
    cc_out = nc.dram_tensor(
        "cc_out", [P, 258], f32, kind="Internal", addr_space="Shared"
    ).ap()

    with tile.TileContext(nc) as tc:
        with (
            tc.tile_pool(name="singles", bufs=1) as singles,
            tc.tile_pool(name="xin", bufs=2) as xin_pool,
            tc.tile_pool(name="sb_small", bufs=4) as sb_small,
            tc.tile_pool(name="sb_ns", bufs=2) as sb_ns,
            tc.tile_pool(name="xo", bufs=4) as xo_pool,
            tc.tile_pool(name="ob", bufs=3) as ob_pool,
            tc.tile_pool(name="ps_s", bufs=1, space="PSUM") as ps_s,
            tc.tile_pool(name="ps_mm", bufs=3, space="PSUM") as ps_mm,
            tc.tile_pool(name="ps_tr", bufs=2, space="PSUM") as ps_tr,
        ):
            # ---- persistent tiles ----
            Zb = singles.tile([P, NPPC], bf16, name="Zb")
            Zp = singles.tile([P, NP_PAD], bf16, name="Zp")
            w_sb = singles.tile([P, 9, P], f32r, name="w_sb")
            cb_sb = singles.tile([P, 1], f32, name="cb_sb")
            lwt_sb = singles.tile([P, P], f32r, name="lwt_sb")
            lb_sb = singles.tile([P, 1], f32, name="lb_sb")
            idf_sb = singles.tile([P, P], f32, name="idf_sb")
            idb_sb = singles.tile([P, P], bf16, name="idb_sb")
            pcd_sb = singles.tile([P, PCHUNKS, P], f32, name="pcd_sb")

            nc.sync.dma_start(w_sb, w9t.rearrange("o k m -> k o m"))
            nc.sync.dma_start(cb_sb, cbv)
            nc.sync.dma_start(lwt_sb, lwt)
            nc.sync.dma_start(lb_sb, lbv)
            nc.sync.dma_start(idf_sb, idf)
            nc.sync.dma_start(idb_sb, idb)
            nc.sync.dma_start(pcd_sb, pcd.rearrange("(t p) c -> p t c", p=P))

            # ---- pcd branch: transpose chunks, linear + ReLU -> Zp [c, pts]
            for t in range(PCHUNKS):
                ptp = ps_tr.tile([P, P], f32, tag="tr", name=f"ptp{t}")
                nc.tensor.transpose(ptp, pcd_sb[:, t, :], idf_sb)
                pt_sb = sb_small.tile([P, P], f32r, tag="pt", name=f"pt{t}")
                nc.vector.tensor_copy(pt_sb, ptp)
                plin = ps_mm.tile([P, P], f32, tag="mm", name=f"plin{t}")
                nc.tensor.matmul(
                    plin,
                    lwt_sb,
                    pt_sb,
                    start=True,
                    stop=True,
                )
                nc.scalar.activation(
                    out=Zp[:, ts(t, P)], in_=plin, func=AF.Relu, bias=lb_sb, scale=1.0
                )
            # zero the padded tail so it cannot pollute covariance sums
            nc.vector.memset(Zp[:, NPPC_P:NP_PAD], 0.0)

            # ---- conv: 9 shifted matmuls per row-chunk, fp32r full-rate ----
            n_chunks = RPC // ROW_CHUNK
            for ci in range(n_chunks):
                xin = xin_pool.tile([P, ROW_CHUNK + 2, W + 2], f32r, tag="xin")
                nc.sync.dma_start(
                    xin, img[:, ci * ROW_CHUNK : ci * ROW_CHUNK + ROW_CHUNK + 2, :]
                )
                for r in range(ROW_CHUNK):
                    row = ci * ROW_CHUNK + r
                    for half in range(2):
                        x0 = half * CONV_N
                        pc = ps_mm.tile([P, CONV_N], f32, tag="mm", name="convps")
                        for o in range(9):
                            ky, kx = divmod(o, 3)
                            nc.tensor.matmul(
                                pc,
                                w_sb[:, o, :],
                                xin[:, r + ky, x0 + kx : x0 + kx + CONV_N],
                                start=(o == 0),
                                stop=(o == 8),
                            )
                        nc.scalar.activation(
                            out=Zb[:, row * W + x0 : row * W + x0 + CONV_N],
                            in_=pc,
                            func=AF.Relu,
                            bias=cb_sb,
                            scale=1.0,
                        )

            # ---- covariance partial sums: S = sum_chunks Zt @ Zt.T ----
            S_img = ps_s.tile([P, P], f32, tag="s", name="S_img")
            S_pcd = ps_s.tile([P, P], f32, tag="s2", name="S_pcd")

            for t in range(PCHUNKS):
                ztp = ps_tr.tile([P, P], bf16, tag="tr", name=f"ztpp{t}")
                nc.tensor.transpose(ztp, Zp[:, ts(t, P)], idb_sb)
                zp_sb = sb_small.tile([P, P], bf16, tag="zpc", name=f"zpc{t}")
                nc.vector.tensor_copy(zp_sb, ztp)
                nc.tensor.matmul(
                    S_pcd, zp_sb, zp_sb, start=(t == 0), stop=(t == PCHUNKS - 1)
                )

            NIMG_CH = NPPC // P  # 300
            for t in range(NIMG_CH):
                ztp = ps_tr.tile([P, P], bf16, tag="tr", name=f"ztpi{t}")
                nc.tensor.transpose(ztp, Zb[:, ts(t, P)], idb_sb)
                zt_sb = sb_small.tile([P, P], bf16, tag="ztc", name=f"ztc{t}")
                nc.vector.tensor_copy(zt_sb, ztp)
                nc.tensor.matmul(
                    S_img, zt_sb, zt_sb, start=(t == 0), stop=(t == NIMG_CH - 1)
                )

            # row sums (from the same bf16 values used in S)
            rs_parts = singles.tile([P, 6], f32, name="rs_parts")
            NSL = 5
            sl = NPPC // NSL
            for i in range(NSL):
                nc.vector.tensor_reduce(
                    rs_parts[:, i : i + 1],
                    Zb[:, i * sl : (i + 1) * sl],
                    axis=AX.X,
                    op=OP.add,
                )
            nc.vector.tensor_reduce(
                rs_parts[:, 5:6], Zp, axis=AX.X, op=OP.add
            )
            r_img = singles.tile([P, 1], f32, name="r_img")
            r_pcd = singles.tile([P, 1], f32, name="r_pcd")
            nc.vector.tensor_reduce(r_img, rs_parts[:, 0:5], axis=AX.X, op=OP.add)
            nc.vector.tensor_copy(r_pcd, rs_parts[:, 5:6])

            # ---- AllReduce the 128x258 partials ----
            ar_sb = singles.tile([P, 258], f32, name="ar_sb")
            nc.vector.tensor_copy(ar_sb[:, 0:P], S_img)
            nc.vector.tensor_copy(ar_sb[:, P : 2 * P], S_pcd)
            nc.vector.tensor_copy(ar_sb[:, 256:257], r_img)
            nc.vector.tensor_copy(ar_sb[:, 257:258], r_pcd)
            nc.sync.dma_start(cc_in, ar_sb)
            nc.gpsimd.collective_compute(
                "AllReduce",
                OP.add,
                replica_groups=[list(range(NCORES))],
                ins=[cc_in],
                outs=[cc_out],
            )
            ars = singles.tile([P, 258], f32, name="ars")
            nc.sync.dma_start(ars, cc_out)

            # ---- replicated small math ----
            def build_cov(S_slice, r_slice, n, name):
                """cov = (S - r r^T / n) / (n-1); A = cov + eps*I."""
                rt_ps = ps_tr.tile([P, P], f32, tag="tr", name=f"rt_{name}")
                nc.tensor.transpose(rt_ps[:1, :], r_slice, idf_sb)
                rT = sb_small.tile([P, P], f32r, tag="rT", name=f"rT_{name}")
                nc.vector.tensor_copy(rT[:1, :], rt_ps[:1, :])
                outer = ps_mm.tile([P, P], f32, tag="mm", name=f"outer_{name}")
                nc.tensor.matmul(
                    outer,
                    rT[:1, :],
                    rT[:1, :],
                    start=True,
                    stop=True,
                )
                t1 = sb_ns.tile([P, P], f32, tag="t1", name=f"t1_{name}")
                nc.vector.tensor_scalar_mul(t1, outer, 1.0 / (n * (n - 1.0)))
                cov = singles.tile([P, P], f32r, name=f"cov_{name}")
                nc.vector.scalar_tensor_tensor(
                    cov, S_slice, 1.0 / (n - 1.0), t1, op0=OP.mult, op1=OP.subtract
                )
                A = singles.tile([P, P], f32r, name=f"A_{name}")
                nc.vector.scalar_tensor_tensor(
                    A, idf_sb, EYE_EPS, cov, op0=OP.mult, op1=OP.add
                )
                return cov, A

            cov_i, A_i = build_cov(ars[:, 0:P], ars[:, 256:257], float(NPIX), "i")
            cov_p, A_p = build_cov(ars[:, P : 2 * P], ars[:, 257:258], float(NP), "p")

            def ns_inverse(A, name):
                """Newton-Schulz inverse of SPD A (symmetric throughout)."""
                rowabs = sb_small.tile([P, 1], f32, tag="ra", name=f"ra_{name}")
                nc.vector.tensor_reduce(
                    rowabs, A, axis=AX.X, op=OP.add, apply_absolute_value=True
                )
                s_all = sb_small.tile([P, 1], f32, tag="sa", name=f"sa_{name}")
                nc.gpsimd.partition_all_reduce(
                    s_all, rowabs, channels=P, reduce_op=bass_isa.ReduceOp.max
                )
                sinv = sb_small.tile([P, 1], f32, tag="si", name=f"si_{name}")
                nc.vector.reciprocal(sinv, s_all)
                X = sb_ns.tile([P, P], f32r, tag=f"X{name}", name=f"X0_{name}")
                nc.vector.tensor_scalar_mul(X, idf_sb, sinv)
                for it in range(NS_ITERS):
                    Yp = ps_mm.tile([P, P], f32, tag="mm", name=f"Y_{name}{it}")
                    nc.tensor.matmul(
                        Yp, A, X, start=True, stop=True
                    )
                    Ys = sb_ns.tile([P, P], f32r, tag=f"Ys{name}", name=f"Ys_{name}{it}")
                    nc.vector.tensor_copy(Ys, Yp)
                    Wp = ps_mm.tile([P, P], f32, tag="mm", name=f"W_{name}{it}")
                    nc.tensor.matmul(
                        Wp, X, Ys, start=True, stop=True
                    )
                    Xn = sb_ns.tile([P, P], f32r, tag=f"X{name}", name=f"Xn_{name}{it}")
                    nc.vector.scalar_tensor_tensor(
                        Xn, X, 2.0, Wp, op0=OP.mult, op1=OP.subtract
                    )
                    X = Xn
                return X

            Xi = ns_inverse(A_i, "i")  # inv(cov_img + eps I)
            Xp = ns_inverse(A_p, "p")  # inv(cov_pcd + eps I)

            def softmax128(mm_lhsT, mm_rhs, name):
                """softmax((lhsT^T@rhs)/128, rows). Both operands symmetric f32."""
                sc = ps_mm.tile([P, P], f32, tag="mm", name=f"sc_{name}")
                nc.tensor.matmul(
                    sc, mm_lhsT, mm_rhs,
                    start=True, stop=True,
                )
                nm = sb_small.tile([P, 1], f32, tag="nm", name=f"nm_{name}")
                nc.vector.tensor_reduce(nm, sc, axis=AX.X, op=OP.max, negate=True)
                nms = sb_small.tile([P, 1], f32, tag="nms", name=f"nms_{name}")
                nc.vector.tensor_scalar_mul(nms, nm, 1.0 / P)
                ex = sb_ns.tile([P, P], f32, tag="ex", name=f"ex_{name}")
                se = sb_small.tile([P, 1], f32, tag="se", name=f"se_{name}")
                nc.scalar.activation(
                    out=ex, in_=sc, func=AF.Exp, bias=nms, scale=1.0 / P,
                    accum_out=se,
                )
                rse = sb_small.tile([P, 1], f32, tag="rse", name=f"rse_{name}")
                nc.vector.reciprocal(rse, se)
                at = singles.tile([P, P], f32, name=f"at_{name}")
                nc.vector.tensor_scalar_mul(at, ex, rse)
                return at

            at_pi = softmax128(cov_i, Xp, "pi")   # softmax(cov_img@inv_pcd/128)
            at_ip = softmax128(cov_p, Xi, "ip")   # softmax(cov_pcd@inv_img/128)
            at_ii = softmax128(cov_i, cov_i, "ii")
            at_pp = softmax128(cov_p, cov_p, "pp")

            def mix_matrix(at_a, at_b, name):
                """M^T in bf16: M = TAU/2*(at_a + at_b)."""
                m = sb_ns.tile([P, P], f32, tag="m", name=f"m_{name}")
                nc.vector.tensor_add(m, at_a, at_b)
                mb = sb_small.tile([P, P], bf16, tag=f"mb{name}", name=f"mb_{name}")
                nc.vector.tensor_scalar_mul(mb, m, TAU / 2.0)
                mt_ps = ps_tr.tile([P, P], bf16, tag="tr", name=f"mtp_{name}")
                nc.tensor.transpose(mt_ps, mb, idb_sb)
                mt = singles.tile([P, P], bf16, name=f"MT_{name}")
                nc.vector.tensor_copy(mt, mt_ps)
                return mt

            # img uses M_img = TAU/2*(at_ip + at_ii); pcd: TAU/2*(at_pi + at_pp)
            MT_img = mix_matrix(at_ip, at_ii, "img")
            MT_pcd = mix_matrix(at_pi, at_pp, "pcd")

            # ---- phase 3: img rows ----
            for r in range(RPC):
                pa = ps_mm.tile([P, CONV_N], f32, tag="mm", name=f"p3a{r}")
                nc.tensor.matmul(
                    pa, MT_img, Zb[:, r * W : r * W + CONV_N], start=True, stop=True
                )
                pb = ps_mm.tile([P, CONV_N], f32, tag="mm", name=f"p3b{r}")
                nc.tensor.matmul(
                    pb, MT_img, Zb[:, r * W + CONV_N : (r + 1) * W],
                    start=True, stop=True,
                )
                xo = xo_pool.tile([P, W], f32r, tag="xo")
                nc.sync.dma_start(xo, img[:, 1 + r, 1 : W + 1])
                ob = ob_pool.tile([P, W], f32, tag="ob")
                nc.vector.scalar_tensor_tensor(
                    ob[:, :CONV_N], xo[:, :CONV_N], 1.0 - TAU, pa,
                    op0=OP.mult, op1=OP.add,
                )
                nc.vector.scalar_tensor_tensor(
                    ob[:, CONV_N:], xo[:, CONV_N:], 1.0 - TAU, pb,
                    op0=OP.mult, op1=OP.add,
                )
                nc.sync.dma_start(img_out[:, r, :], ob)

            # ---- phase 3: pcd chunks ----
            pcd_out_r = pcd_out.rearrange("(t p) c -> t p c", p=P)
            for t in range(PCHUNKS):
                pp = ps_mm.tile([P, P], f32, tag="mm", name=f"p3p{t}")
                nc.tensor.matmul(pp, Zp[:, ts(t, P)], MT_pcd, start=True, stop=True)
                po = ob_pool.tile([P, P], f32, tag="po")
                nc.vector.scalar_tensor_tensor(
                    po, pcd_sb[:, t, :], 1.0 - TAU, pp, op0=OP.mult, op1=OP.add
                )
                nc.sync.dma_start(pcd_out_r[t], po)

    nc.compile()
    return nc


def _get_program():
    if "nc" not in _cache:
        _cache["nc"] = _build_program()
    return _cache["nc"]


def _prepare_in_maps(inputs):
    img_f = np.asarray(inputs["img_feats_f"], dtype=np.float32)
    pcd_f = np.asarray(inputs["pcd_feats_f"], dtype=np.float32)
    conv_w = np.asarray(inputs["conv_w"], dtype=np.float32)
    conv_b = np.asarray(inputs["conv_b"], dtype=np.float32)
    bn_gamma = np.asarray(inputs["bn_gamma"], dtype=np.float32)
    bn_beta = np.asarray(inputs["bn_beta"], dtype=np.float32)
    bn_mean = np.asarray(inputs["bn_mean"], dtype=np.float32)
    bn_var = np.asarray(inputs["bn_var"], dtype=np.float32)
    lin_w = np.asarray(inputs["lin_w"], dtype=np.float32)
    lin_b = np.asarray(inputs["lin_b"], dtype=np.float32)

    # fold BN (eval mode) into the conv weights/bias
    scale = (bn_gamma.astype(np.float64) / np.sqrt(bn_var.astype(np.float64) + BN_EPS))
    wf = (conv_w.astype(np.float64) * scale[:, None, None, None]).astype(np.float32)
    bf = ((conv_b.astype(np.float64) - bn_mean) * scale + bn_beta).astype(np.float32)

    w9t = np.ascontiguousarray(wf.transpose(2, 3, 1, 0).reshape(9, P, P))
    cb = bf.reshape(P, 1)
    lwt = np.ascontiguousarray(lin_w.T)        # [cin, cout]
    lb = lin_b.reshape(P, 1).astype(np.float32)
    idf = np.eye(P, dtype=np.float32)
    idb = np.eye(P, dtype=ml_dtypes.bfloat16)

    imgp = np.zeros((P, H + 2, W + 2), np.float32)
    imgp[:, 1 : H + 1, 1 : W + 1] = img_f[0]

    in_maps = []
    for k in range(NCORES):
        slab = np.ascontiguousarray(imgp[:, RPC * k : RPC * k + RPC + 2, :])
        shard = np.zeros((NP_PAD, P), np.float32)
        shard[:NPPC_P] = pcd_f[NPPC_P * k : NPPC_P * (k + 1)]
        in_maps.append(
            dict(img=slab, pcd=shard, w9t=w9t, cb=cb, lwt=lwt, lb=lb, idf=idf,
                 idb=idb)
        )
    return in_maps


def _assemble(results):
    img_full = np.concatenate(
        [results[k]["img_out"] for k in range(NCORES)], axis=1
    )  # [128, 480, 640]
    pcd_full = np.concatenate(
        [results[k]["pcd_out"][:NPPC_P] for k in range(NCORES)], axis=0
    )  # [22432, 128]
    return (
        np.ascontiguousarray(img_full[None]).astype(np.float32),
        np.ascontiguousarray(pcd_full).astype(np.float32),
    )


def kernel(**inputs):
    from concourse.bass_utils import run_bass_kernel_spmd

    nc = _get_program()
    in_maps = _prepare_in_maps(inputs)
    res = run_bass_kernel_spmd(nc, in_maps, core_ids=list(range(NCORES)))
    return _assemble(res.results)


# revision 10
# speedup vs baseline: 1.3157x; 1.3157x over previous
"""Trainium2 Bass kernel for nn_CovDiFF_full_40312563040970.

Pipeline (per the reference model):
  img branch: 3x3 conv (+folded BN) + ReLU -> Z  [128, 307200]
  pcd branch: linear + ReLU               -> Zp [128, 22432]
  cov_img = cov(Z), cov_pcd = cov(Zp)  (torch.cov semantics)
  attention matrices from covariances (inverse + row-softmax), then
  out = TAU/2*(at_a + at_b) @ Z  + (1-TAU)*input   for each branch.

Distribution: 8 NeuronCores; image rows and points sharded across cores.
Each core computes a partial covariance (sum-of-outer-products + row sums),
a 128x258 AllReduce combines them, and the tiny 128x128 math (Newton-Schulz
inverse, softmax) is replicated on every core.  The heavy matmuls run in
fp16 (m10) on the TensorEngine; covariance chunk-transposes and their
accumulating matmuls are fused into the conv loop so the PE sees one dense
instruction stream.
"""

import numpy as np

P = 128
H, W = 480, 640
NCORES = 8
RPC = H // NCORES            # 60 output rows per core
NPPC = RPC * W               # 38400 pixels per core
NPIX = H * W                 # 307200 pixels total
NP = 22432                   # points total
NPPC_P = NP // NCORES        # 2804 points per core
PCHUNKS = 22                 # point chunks of 128 (padded)
NP_PAD = PCHUNKS * P         # 2816
TAU = 0.2
BN_EPS = 1e-5
EYE_EPS = 1e-6
NS_ITERS = 12
ROW_CHUNK = 10               # conv: output rows per input DMA chunk
CONV_N = 320                 # conv matmul free dim (640 = 2x320)

_cache = {}


def _build_program():
    import concourse.bacc as bacc
    import concourse.tile as tile
    import concourse.mybir as mybir
    from concourse import bass_isa
    from concourse.bass import ts

    f32 = mybir.dt.float32
    f32r = mybir.dt.float32r
    f16 = mybir.dt.float16
    AX = mybir.AxisListType
    OP = mybir.AluOpType
    AF = mybir.ActivationFunctionType

    nc = bacc.Bacc(
        "TRN2", target_bir_lowering=False, debug=False, num_devices=NCORES
    )

    img = nc.dram_tensor("img", [P, RPC + 2, W + 2], f16, kind="ExternalInput").ap()
    pcd = nc.dram_tensor("pcd", [NP_PAD, P], f16, kind="ExternalInput").ap()
    w9t = nc.dram_tensor("w9t", [9, P, P], f16, kind="ExternalInput").ap()
    cbv = nc.dram_tensor("cb", [P, 1], f32, kind="ExternalInput").ap()
    lwt = nc.dram_tensor("lwt", [P, P], f16, kind="ExternalInput").ap()
    lbv = nc.dram_tensor("lb", [P, 1], f32, kind="ExternalInput").ap()
    idf = nc.dram_tensor("idf", [P, P], f32, kind="ExternalInput").ap()
    idh = nc.dram_tensor("idh", [P, P], f16, kind="ExternalInput").ap()

    img_out = nc.dram_tensor("img_out", [P, RPC, W], f32, kind="ExternalOutput").ap()
    pcd_out = nc.dram_tensor("pcd_out", [NP_PAD, P], f32, kind="ExternalOutput").ap()

    cc_in = nc.dram_tensor("cc_in", [P, 258], f32, kind="Internal").ap()
    cc_out = nc.dram_tensor(
        "cc_out", [P, 258], f32, kind="Internal", addr_space="Shared"
    ).ap()

    with tile.TileContext(nc) as tc:
        with (
            tc.tile_pool(name="singles", bufs=1) as singles,
            tc.tile_pool(name="xin", bufs=3) as xin_pool,
            tc.tile_pool(name="sb_small", bufs=4) as sb_small,
            tc.tile_pool(name="sb_ns", bufs=2) as sb_ns,
            tc.tile_pool(name="xo", bufs=4) as xo_pool,
            tc.tile_pool(name="ob", bufs=4) as ob_pool,
            tc.tile_pool(name="ps_s", bufs=1, space="PSUM") as ps_s,
            tc.tile_pool(name="ps_mm", bufs=4, space="PSUM") as ps_mm,
            tc.tile_pool(name="ps_tr", bufs=2, space="PSUM") as ps_tr,
        ):
            # ---- persistent tiles ----
            Zb = singles.tile([P, NPPC], f16, name="Zb")
            Zp = singles.tile([P, NP_PAD], f16, name="Zp")
            w_sb = singles.tile([P, 9, P], f16, name="w_sb")
            cb_sb = singles.tile([P, 1], f32, name="cb_sb")
            lwt_sb = singles.tile([P, P], f16, name="lwt_sb")
            lb_sb = singles.tile([P, 1], f32, name="lb_sb")
            idf_sb = singles.tile([P, P], f32, name="idf_sb")
            idh_sb = singles.tile([P, P], f16, name="idh_sb")
            pcd_sb = singles.tile([P, PCHUNKS, P], f16, name="pcd_sb")

            nc.sync.dma_start(w_sb, w9t.rearrange("o k m -> k o m"))
            nc.sync.dma_start(cb_sb, cbv)
            nc.sync.dma_start(lwt_sb, lwt)
            nc.sync.dma_start(lb_sb, lbv)
            nc.sync.dma_start(idf_sb, idf)
            nc.sync.dma_start(idh_sb, idh)
            nc.sync.dma_start(pcd_sb, pcd.rearrange("(t p) c -> p t c", p=P))

            S_img = ps_s.tile([P, P], f32, tag="s", name="S_img")
            S_pcd = ps_s.tile([P, P], f32, tag="s2", name="S_pcd")

            # ---- pcd branch: transpose chunks, linear + ReLU -> Zp [c, pts]
            for t in range(PCHUNKS):
                ptp = ps_tr.tile([P, P], f16, tag="tr", name=f"ptp{t}")
                nc.tensor.transpose(ptp, pcd_sb[:, t, :], idh_sb)
                pt_sb = sb_small.tile([P, P], f16, tag="pt", name=f"pt{t}")
                nc.vector.tensor_copy(pt_sb, ptp)
                plin = ps_mm.tile([P, P], f32, tag="mm", name=f"plin{t}")
                nc.tensor.matmul(plin, lwt_sb, pt_sb, start=True, stop=True)
                nc.scalar.activation(
                    out=Zp[:, ts(t, P)], in_=plin, func=AF.Relu, bias=lb_sb, scale=1.0
                )
            # zero the padded tail so it cannot pollute covariance sums
            nc.vector.memset(Zp[:, NPPC_P:NP_PAD], 0.0)

            # cov_pcd partial accumulation
            for t in range(PCHUNKS):
                ztp = ps_tr.tile([P, P], f16, tag="tr", name=f"ztpp{t}")
                nc.tensor.transpose(ztp, Zp[:, ts(t, P)], idh_sb)
                zp_sb = sb_small.tile([P, P], f16, tag="zpc", name=f"zpc{t}")
                nc.any.tensor_copy(out=zp_sb, in_=ztp)
                nc.tensor.matmul(
                    S_pcd, zp_sb, zp_sb, start=(t == 0), stop=(t == PCHUNKS - 1)
                )

            # ---- conv (9 shifted fp16 matmuls) fused with cov_img chunks ----
            rs_parts = singles.tile([P, 8], f32, name="rs_parts")
            n_chunks = RPC // ROW_CHUNK
            CPR = W // P  # cov chunks per row (5)
            for ci in range(n_chunks):
                xin = xin_pool.tile([P, ROW_CHUNK + 2, W + 2], f16, tag="xin")
                nc.sync.dma_start(
                    xin, img[:, ci * ROW_CHUNK : ci * ROW_CHUNK + ROW_CHUNK + 2, :]
                )
                for r in range(ROW_CHUNK):
                    row = ci * ROW_CHUNK + r
                    for half in range(2):
                        x0 = half * CONV_N
                        pc = ps_mm.tile([P, CONV_N], f32, tag="mm", name="convps")
                        for o in range(9):
                            ky, kx = divmod(o, 3)
                            nc.tensor.matmul(
                                pc,
                                w_sb[:, o, :],
                                xin[:, r + ky, x0 + kx : x0 + kx + CONV_N],
                                start=(o == 0),
                                stop=(o == 8),
                            )
                        nc.scalar.activation(
                            out=Zb[:, row * W + x0 : row * W + x0 + CONV_N],
                            in_=pc,
                            func=AF.Relu,
                            bias=cb_sb,
                            scale=1.0,
                        )
                    # cov_img chunks for this row (5 x 128 px)
                    for j in range(CPR):
                        t = row * CPR + j
                        ztp = ps_tr.tile([P, P], f16, tag="tr", name=f"ztpi{t}")
                        nc.tensor.transpose(
                            ztp, Zb[:, row * W + j * P : row * W + (j + 1) * P],
                            idh_sb,
                        )
                        zt_sb = sb_small.tile([P, P], f16, tag="ztc", name=f"ztc{t}")
                        nc.any.tensor_copy(out=zt_sb, in_=ztp)
                        nc.tensor.matmul(
                            S_img,
                            zt_sb,
                            zt_sb,
                            start=(t == 0),
                            stop=(t == RPC * CPR - 1),
                        )
                # partial row-sum for this chunk (overlaps with next chunk)
                nc.vector.tensor_reduce(
                    rs_parts[:, ci : ci + 1],
                    Zb[:, ci * ROW_CHUNK * W : (ci + 1) * ROW_CHUNK * W],
                    axis=AX.X,
                    op=OP.add,
                )

            nc.vector.tensor_reduce(rs_parts[:, 6:7], Zp, axis=AX.X, op=OP.add)
            r_img = singles.tile([P, 1], f32, name="r_img")
            nc.vector.tensor_reduce(r_img, rs_parts[:, 0:6], axis=AX.X, op=OP.add)

            # ---- AllReduce the 128x258 partials ----
            ar_sb = singles.tile([P, 258], f32, name="ar_sb")
            nc.vector.tensor_copy(ar_sb[:, 0:P], S_img)
            nc.vector.tensor_copy(ar_sb[:, P : 2 * P], S_pcd)
            nc.vector.tensor_copy(ar_sb[:, 256:257], r_img)
            nc.vector.tensor_copy(ar_sb[:, 257:258], rs_parts[:, 6:7])
            nc.sync.dma_start(cc_in, ar_sb)
            nc.gpsimd.collective_compute(
                "AllReduce",
                OP.add,
                replica_groups=[list(range(NCORES))],
                ins=[cc_in],
                outs=[cc_out],
            )
            ars = singles.tile([P, 258], f32, name="ars")
            nc.sync.dma_start(ars, cc_out)

            # ---- replicated small math ----
            def build_cov(S_slice, r_slice, n, name):
                """cov = (S - r r^T / n) / (n-1)  (fp16); A = cov + eps*I."""
                rt_ps = ps_tr.tile([P, P], f32, tag="tr", name=f"rt_{name}")
                nc.tensor.transpose(rt_ps[:1, :], r_slice, idf_sb)
                rT = sb_small.tile([P, P], f32r, tag="rT", name=f"rT_{name}")
                nc.vector.tensor_copy(rT[:1, :], rt_ps[:1, :])
                outer = ps_mm.tile([P, P], f32, tag="mm", name=f"outer_{name}")
                nc.tensor.matmul(
                    outer, rT[:1, :], rT[:1, :], start=True, stop=True
                )
                t1 = sb_ns.tile([P, P], f32, tag="t1", name=f"t1_{name}")
                nc.vector.tensor_scalar_mul(t1, outer, 1.0 / (n * (n - 1.0)))
                cov = singles.tile([P, P], f16, name=f"cov_{name}")
                nc.vector.scalar_tensor_tensor(
                    cov, S_slice, 1.0 / (n - 1.0), t1, op0=OP.mult, op1=OP.subtract
                )
                A = singles.tile([P, P], f16, name=f"A_{name}")
                nc.vector.scalar_tensor_tensor(
                    A, idf_sb, EYE_EPS, cov, op0=OP.mult, op1=OP.add
                )
                return cov, A

            cov_i, A_i = build_cov(ars[:, 0:P], ars[:, 256:257], float(NPIX), "i")
            cov_p, A_p = build_cov(ars[:, P : 2 * P], ars[:, 257:258], float(NP), "p")

            def ns_init(A, name):
                rowabs = sb_small.tile([P, 1], f32, tag="ra", name=f"ra_{name}")
                nc.vector.tensor_reduce(
                    rowabs, A, axis=AX.X, op=OP.add, apply_absolute_value=True
                )
                s_all = sb_small.tile([P, 1], f32, tag="sa", name=f"sa_{name}")
                nc.gpsimd.partition_all_reduce(
                    s_all, rowabs, channels=P, reduce_op=bass_isa.ReduceOp.max
                )
                sinv = sb_small.tile([P, 1], f32, tag="si", name=f"si_{name}")
                nc.vector.reciprocal(sinv, s_all)
                X = sb_ns.tile([P, P], f16, tag=f"X{name}", name=f"X0_{name}")
                nc.vector.tensor_scalar_mul(X, idf_sb, sinv)
                return X

            def ns_step(A, X, name, it):
                Yp = ps_mm.tile([P, P], f32, tag="mm", name=f"Y_{name}{it}")
                nc.tensor.matmul(Yp, A, X, start=True, stop=True)
                Ys = sb_ns.tile([P, P], f16, tag=f"Ys{name}", name=f"Ys_{name}{it}")
                nc.vector.tensor_copy(Ys, Yp)
                Wp = ps_mm.tile([P, P], f32, tag="mm", name=f"W_{name}{it}")
                nc.tensor.matmul(Wp, X, Ys, start=True, stop=True)
                Xn = sb_ns.tile([P, P], f16, tag=f"X{name}", name=f"Xn_{name}{it}")
                nc.vector.scalar_tensor_tensor(
                    Xn, X, 2.0, Wp, op0=OP.mult, op1=OP.subtract
                )
                return Xn

            Xi = ns_init(A_i, "i")
            Xp = ns_init(A_p, "p")
            for it in range(NS_ITERS):
                Xi = ns_step(A_i, Xi, "i", it)
                Xp = ns_step(A_p, Xp, "p", it)

            def softmax128(mm_lhsT, mm_rhs, name):
                """softmax((lhsT^T@rhs)/128, rows). Operands symmetric fp16."""
                sc = ps_mm.tile([P, P], f32, tag="mm", name=f"sc_{name}")
                nc.tensor.matmul(sc, mm_lhsT, mm_rhs, start=True, stop=True)
                nm = sb_small.tile([P, 1], f32, tag="nm", name=f"nm_{name}")
                nc.vector.tensor_reduce(nm, sc, axis=AX.X, op=OP.max, negate=True)
                nms = sb_small.tile([P, 1], f32, tag="nms", name=f"nms_{name}")
                nc.vector.tensor_scalar_mul(nms, nm, 1.0 / P)
                ex = sb_ns.tile([P, P], f32, tag="ex", name=f"ex_{name}")
                se = sb_small.tile([P, 1], f32, tag="se", name=f"se_{name}")
                nc.scalar.activation(
                    out=ex, in_=sc, func=AF.Exp, bias=nms, scale=1.0 / P,
                    accum_out=se,
                )
                rse = sb_small.tile([P, 1], f32, tag="rse", name=f"rse_{name}")
                nc.vector.reciprocal(rse, se)
                at = singles.tile([P, P], f32, name=f"at_{name}")
                nc.vector.tensor_scalar_mul(at, ex, rse)
                return at

            at_pi = softmax128(cov_i, Xp, "pi")   # softmax(cov_img@inv_pcd/128)
            at_ip = softmax128(cov_p, Xi, "ip")   # softmax(cov_pcd@inv_img/128)
            at_ii = softmax128(cov_i, cov_i, "ii")
            at_pp = softmax128(cov_p, cov_p, "pp")

            def mix_matrix(at_a, at_b, name):
                """M^T in fp16: M = TAU/2*(at_a + at_b)."""
                m = sb_ns.tile([P, P], f32, tag="m", name=f"m_{name}")
                nc.vector.tensor_add(m, at_a, at_b)
                mb = sb_small.tile([P, P], f16, tag=f"mb{name}", name=f"mb_{name}")
                nc.vector.tensor_scalar_mul(mb, m, TAU / 2.0)
                mt_ps = ps_tr.tile([P, P], f16, tag="tr", name=f"mtp_{name}")
                nc.tensor.transpose(mt_ps, mb, idh_sb)
                mt = singles.tile([P, P], f16, name=f"MT_{name}")
                nc.vector.tensor_copy(mt, mt_ps)
                return mt

            MT_img = mix_matrix(at_ip, at_ii, "img")
            MT_pcd = mix_matrix(at_pi, at_pp, "pcd")

            # ---- phase 3: pcd chunks then img rows ----
            pcd_out_r = pcd_out.rearrange("(t p) c -> t p c", p=P)
            for t in range(PCHUNKS):
                pp = ps_mm.tile([P, P], f32, tag="mm", name=f"p3p{t}")
                nc.tensor.matmul(pp, Zp[:, ts(t, P)], MT_pcd, start=True, stop=True)
                po = ob_pool.tile([P, P], f32, tag="po")
                nc.vector.scalar_tensor_tensor(
                    po, pcd_sb[:, t, :], 1.0 - TAU, pp, op0=OP.mult, op1=OP.add
                )
                nc.sync.dma_start(pcd_out_r[t], po)

            for r in range(RPC):
                pa = ps_mm.tile([P, CONV_N], f32, tag="mm", name=f"p3a{r}")
                nc.tensor.matmul(
                    pa, MT_img, Zb[:, r * W : r * W + CONV_N], start=True, stop=True
                )
                pb = ps_mm.tile([P, CONV_N], f32, tag="mm", name=f"p3b{r}")
                nc.tensor.matmul(
                    pb, MT_img, Zb[:, r * W + CONV_N : (r + 1) * W],
                    start=True, stop=True,
                )
                xo = xo_pool.tile([P, W], f16, tag="xo")
                nc.sync.dma_start(xo, img[:, 1 + r, 1 : W + 1])
                ob = ob_pool.tile([P, W], f32, tag="ob")
                nc.vector.scalar_tensor_tensor(
                    ob[:, :CONV_N], xo[:, :CONV_N], 1.0 - TAU, pa,
                    op0=OP.mult, op1=OP.add,
                )
                nc.vector.scalar_tensor_tensor(
                    ob[:, CONV_N:], xo[:, CONV_N:], 1.0 - TAU, pb,
                    op0=OP.mult, op1=OP.add,
                )
                nc.sync.dma_start(img_out[:, r, :], ob)

    nc.compile()
    return nc


def _get_program():
    if "nc" not in _cache:
        _cache["nc"] = _build_program()
    return _cache["nc"]


def _prepare_in_maps(inputs):
    img_f = np.asarray(inputs["img_feats_f"], dtype=np.float32)
    pcd_f = np.asarray(inputs["pcd_feats_f"], dtype=np.float32)
    conv_w = np.asarray(inputs["conv_w"], dtype=np.float32)
    conv_b = np.asarray(inputs["conv_b"], dtype=np.float32)
    bn_gamma = np.asarray(inputs["bn_gamma"], dtype=np.float32)
    bn_beta = np.asarray(inputs["bn_beta"], dtype=np.float32)
    bn_mean = np.asarray(inputs["bn_mean"], dtype=np.float32)
    bn_var = np.asarray(inputs["bn_var"], dtype=np.float32)
    lin_w = np.asarray(inputs["lin_w"], dtype=np.float32)
    lin_b = np.asarray(inputs["lin_b"], dtype=np.float32)

    # fold BN (eval mode) into the conv weights/bias
    scale = (bn_gamma.astype(np.float64) / np.sqrt(bn_var.astype(np.float64) + BN_EPS))
    wf = (conv_w.astype(np.float64) * scale[:, None, None, None]).astype(np.float32)
    bf = ((conv_b.astype(np.float64) - bn_mean) * scale + bn_beta).astype(np.float32)

    w9t = np.ascontiguousarray(
        wf.transpose(2, 3, 1, 0).reshape(9, P, P)
    ).astype(np.float16)
    cb = bf.reshape(P, 1)
    lwt = np.ascontiguousarray(lin_w.T).astype(np.float16)   # [cin, cout]
    lb = lin_b.reshape(P, 1).astype(np.float32)
    idf = np.eye(P, dtype=np.float32)
    idh = np.eye(P, dtype=np.float16)

    imgp = np.zeros((P, H + 2, W + 2), np.float16)
    imgp[:, 1 : H + 1, 1 : W + 1] = img_f[0].astype(np.float16)

    in_maps = []
    for k in range(NCORES):
        slab = np.ascontiguousarray(imgp[:, RPC * k : RPC * k + RPC + 2, :])
        shard = np.zeros((NP_PAD, P), np.float16)
        shard[:NPPC_P] = pcd_f[NPPC_P * k : NPPC_P * (k + 1)].astype(np.float16)
        in_maps.append(
            dict(img=slab, pcd=shard, w9t=w9t, cb=cb, lwt=lwt, lb=lb, idf=idf,
                 idh=idh)
        )
    return in_maps


def _assemble(results):
    img_full = np.concatenate(
        [results[k]["img_out"] for k in range(NCORES)], axis=1
    )  # [128, 480, 640]
    pcd_full = np.concatenate(
        [results[k]["pcd_out"][:NPPC_P] for k in range(NCORES)], axis=0
    )  # [22432, 128]
    return (
        np.ascontiguousarray(img_full[None]).astype(np.float32),
        np.ascontiguousarray(pcd_full).astype(np.float32),
    )


def kernel(**inputs):
    from concourse.bass_utils import run_bass_kernel_spmd

    nc = _get_program()
    in_maps = _prepare_in_maps(inputs)
    res = run_bass_kernel_spmd(nc, in_maps, core_ids=list(range(NCORES)))
    return _assemble(res.results)
